# revision 34
# baseline (speedup 1.0000x reference)
"""Multi-head causal attention (B=4, S=2048, D=1024, H=16) on 8 Trainium2 cores.

Sharding: core c -> (batch b = c//2, head-half hh = c%2): each core computes
attention for one batch and 8 of the 16 heads plus the partial output
projection against its row-shard of Wo. Host sums the per-batch core pair
(the Wo row-shard all-reduce) and transposes.

On-device schedule (per core; matmul operands bf16, accumulation fp32):
  - scores are computed transposed (S^T tiles: keys on partitions) so exp on
    ScalarE writes P^T directly; softmax denominators come free from a
    ones-column appended to V (row 64 of the P^T @ V_aug product).
  - work is organized as 16 "units" = (query-group g4 of 512 tokens) x
    (head-pair g), swept g4-major. Each unit runs windows of <=1024 score
    columns: PE score matmuls -> ScalarE exp -> (diag-tile mask on DVE).
  - the AV accumulation of unit i is interleaved between the score windows
    of unit i+1, and projection / output-projection blocks are fed one per
    window as PE filler, so the PE never idles waiting for ScalarE (idle PE
    drops out of its high p-state and halves matmul throughput).
  - softmax normalization: reciprocal of the denominator row on DVE,
    partition-broadcast on GpSimd, multiply on DVE - no DMA round trips.
  - output projection is emitted per (128-row d chunk, 512-col s chunk) as
    soon as the oT columns for that s chunk are complete, keeping only the
    last s-group's projection on the critical-path tail.
"""

import os
import sys
from collections import deque
from contextlib import ExitStack

for _p in (
    "/opt/trn_rl_repo/concourse",
    "/root/.axon_site/_ro/trn_rl_repo/concourse",
):
    if os.path.isdir(_p) and _p not in sys.path:
        sys.path.append(_p)

import numpy as np
import ml_dtypes

BF16 = ml_dtypes.bfloat16

HD = 64          # head dim
NH = 8           # heads per core
G = NH // 2      # head-pair groups (2 heads -> 128 partitions)
EC = NH * HD // 128  # o^T feature chunks (=4)


def g4_geom(g4):
    """Blocks/windows for query group g4 (queries [512*g4, 512*(g4+1))).

    blocks: (t, q_lo, n) - key tile t contributes P^T columns for queries
    [q_lo, q_lo+n). off[t]: column offset of tile t's block in the packed
    pts tile. Windows greedily pack consecutive blocks to <=1024 columns;
    block offsets within a window never straddle a 512 (PSUM bank) boundary
    by construction (full 512 blocks, then a 512/384/256/128 tail).
    """
    blocks = []
    for t in range(4 * g4 + 4):
        if t < 4 * g4:
            q_lo, n = g4 * 512, 512
        else:
            q_lo = t * 128
            n = (g4 + 1) * 512 - q_lo
        blocks.append((t, q_lo, n))
    off = {}
    c = 0
    for t, _, n in blocks:
        off[t] = c
        c += n
    # greedy-pack consecutive blocks into <=1024-col windows (fewer, larger
    # exp instructions); score matmuls use one start/stop pair per PSUM bank
    wins = []
    cur, cc = [], 0
    for b in blocks:
        if cc + b[2] > 1024:
            wins.append(cur)
            cur, cc = [], 0
        cur.append(b)
        cc += b[2]
    if cur:
        wins.append(cur)
    return blocks, off, c, wins


def build_nc(S, D):
    import concourse.tile as tile
    from concourse import bacc, mybir

    f32 = mybir.dt.float32
    bf16 = mybir.dt.bfloat16
    Exp = mybir.ActivationFunctionType.Exp
    add = mybir.AluOpType.add
    mult = mybir.AluOpType.mult

    KC = D // 128    # contraction chunks over model dim
    ST = S // 128    # 128-token tiles
    SC = S // 512    # 512-token chunks

    nc = bacc.Bacc(None, target_bir_lowering=False)

    xq = nc.dram_tensor("xq", [D, S], bf16, kind="ExternalInput")
    xk = nc.dram_tensor("xk", [D, S], bf16, kind="ExternalInput")
    xv = nc.dram_tensor("xv", [D, S], bf16, kind="ExternalInput")
    wq = nc.dram_tensor("wq", [D, NH * HD], bf16, kind="ExternalInput")
    wk = nc.dram_tensor("wk", [D, NH * HD], bf16, kind="ExternalInput")
    wv = nc.dram_tensor("wv", [D, NH * HD], bf16, kind="ExternalInput")
    wo = nc.dram_tensor("wo", [NH * HD, D], bf16, kind="ExternalInput")
    bqd = nc.dram_tensor("bq", [128, G], f32, kind="ExternalInput")
    bkd = nc.dram_tensor("bk", [128, G], f32, kind="ExternalInput")
    bvd = nc.dram_tensor("bv", [128, NH, HD], f32, kind="ExternalInput")
    bod = nc.dram_tensor("bo", [128, D // 128], f32, kind="ExternalInput")
    maskd = nc.dram_tensor("mask", [128, 128], bf16, kind="ExternalInput")
    out = nc.dram_tensor("out", [D, S], f32, kind="ExternalOutput")

    with tile.TileContext(nc) as tc, ExitStack() as ctx:
        const_pool = ctx.enter_context(tc.tile_pool(name="const", bufs=1))
        xpool = ctx.enter_context(tc.tile_pool(name="x", bufs=3))
        qk_pool = ctx.enter_context(tc.tile_pool(name="qk", bufs=1))
        v_pool = ctx.enter_context(tc.tile_pool(name="v", bufs=1))
        pt_pool = ctx.enter_context(tc.tile_pool(name="pt", bufs=2))
        o_pool = ctx.enter_context(tc.tile_pool(name="o", bufs=1))
        r_pool = ctx.enter_context(tc.tile_pool(name="r", bufs=2))
        ou_pool = ctx.enter_context(tc.tile_pool(name="ou", bufs=2))
        ps_st = ctx.enter_context(tc.tile_pool(name="psst", bufs=2, space="PSUM"))
        ps_av = ctx.enter_context(tc.tile_pool(name="psav", bufs=2, space="PSUM"))
        ps_sm = ctx.enter_context(tc.tile_pool(name="pssm", bufs=2, space="PSUM"))

        # ---- persistent SBUF tensors ----
        wq_sb = const_pool.tile([128, KC, NH * HD], bf16)
        wk_sb = const_pool.tile([128, KC, NH * HD], bf16)
        wv_sb = const_pool.tile([128, KC, NH * HD], bf16)
        wo_sb = const_pool.tile([128, EC, D], bf16)
        bq_sb = const_pool.tile([128, G], f32)
        bk_sb = const_pool.tile([128, G], f32)
        bv_sb = const_pool.tile([128, NH, HD], f32)
        bo_sb = const_pool.tile([128, D // 128], f32)
        mask_sb = const_pool.tile([128, 128], bf16)

        qT = qk_pool.tile([128, G, S], bf16, tag="qT")
        kT = qk_pool.tile([128, G, S], bf16, tag="kT")
        v_sb = v_pool.tile([128, ST, NH, HD + 1], bf16, tag="v")
        oT = o_pool.tile([128, EC, S], bf16, tag="oT")

        # ---- startup DMAs: pair-0 q/k weights + first x slices first ----
        def wrearr(w):
            return w.rearrange("(kc p) m -> p kc m", p=128)

        nc.sync.dma_start(wq_sb[:, :, 0:128], wrearr(wq[:, 0:128]))
        xcache = {}

        def get_x(kind, sc):
            key = (kind, sc)
            if key not in xcache:
                xt = xpool.tile([128, KC, 512], bf16, tag="xb", name=f"x{kind}{sc}")
                src = {"q": xq, "k": xk, "v": xv}[kind]
                nc.sync.dma_start(
                    xt[:],
                    src.rearrange("(kc p) s -> p kc s", p=128)[
                        :, :, sc * 512 : (sc + 1) * 512
                    ],
                )
                xcache[key] = xt
            return xcache[key]

        get_x("q", 0)
        nc.sync.dma_start(bq_sb[:], bqd[:])

        # ones columns of V_aug (softmax denominator trick)
        nc.vector.memset(v_sb[:, :, :, HD : HD + 1], 1.0)

        # emitted-work accounting (ns) for deficit-paced filler: keep the PE
        # queue fed just ahead of the ScalarE exp pace, so early windows don't
        # over-consume filler and starve the late sweeps into p-state drops
        acc = {"pe": 0.0, "sc": 0.0}

        def qk_block(kind, g, sc):
            """One 512-col projection chunk for pair g of q or k."""
            xt = get_x(kind, sc)
            wsb = wq_sb if kind == "q" else wk_sb
            dst = qT if kind == "q" else kT
            bsb = bq_sb if kind == "q" else bk_sb
            psum = ps_sm.tile([128, 512], f32, tag="mm", name=f"p{kind}{g}_{sc}")
            for c in range(KC):
                nc.tensor.matmul(
                    psum[:],
                    wsb[:, c, g * 128 : (g + 1) * 128],
                    xt[:, c, :],
                    start=(c == 0),
                    stop=(c == KC - 1),
                )
            nc.vector.tensor_scalar_add(
                dst[:, g, sc * 512 : (sc + 1) * 512], psum[:], bsb[:, g : g + 1]
            )
            acc["pe"] += 1750

        def v_block(sc, qi):
            """V projection for tokens [sc*512 + qi*128, +128)."""
            xt = get_x("v", sc)
            psum = ps_sm.tile([128, NH, HD], f32, tag="mm", name=f"pv{sc}_{qi}")
            for c in range(KC):
                nc.tensor.matmul(
                    psum[:],
                    xt[:, c, qi * 128 : (qi + 1) * 128],
                    wv_sb[:, c, :],
                    start=(c == 0),
                    stop=(c == KC - 1),
                )
            nc.vector.tensor_tensor(
                v_sb[:, sc * 4 + qi, :, 0:HD], psum[:], bv_sb[:], add
            )
            acc["pe"] += 1750

        def op_block(dc, sc):
            """Output projection chunk out[dc*128:+128, sc*512:+512]."""
            psum = ps_sm.tile([128, 512], f32, tag="mm", name=f"po{dc}_{sc}")
            for c in range(EC):
                nc.tensor.matmul(
                    psum[:],
                    wo_sb[:, c, dc * 128 : (dc + 1) * 128],
                    oT[:, c, sc * 512 : (sc + 1) * 512],
                    start=(c == 0),
                    stop=(c == EC - 1),
                )
            ot = ou_pool.tile([128, 512], f32, tag="ot", name=f"ot{dc}_{sc}")
            nc.vector.tensor_scalar_add(ot[:], psum[:], bo_sb[:, dc : dc + 1])
            nc.sync.dma_start(
                out[dc * 128 : (dc + 1) * 128, sc * 512 : (sc + 1) * 512], ot[:]
            )
            acc["pe"] += 880

        # upfront: pair-0 q/k projection for the first query/key chunk
        qk_block("q", 0, 0)
        nc.sync.dma_start(wk_sb[:, :, 0:128], wrearr(wk[:, 0:128]))
        get_x("k", 0)
        nc.sync.dma_start(bk_sb[:], bkd[:])
        # V-path loads early: the first AV (carried into unit (0,1)) needs
        # the sweep-0 V projection done by ~t+10us
        nc.sync.dma_start(wv_sb[:], wrearr(wv))
        get_x("v", 0)
        nc.sync.dma_start(bv_sb[:], bvd[:])
        qk_block("k", 0, 0)
        nc.sync.dma_start(mask_sb[:], maskd[:])
        # remaining weight/bias loads (consumed by filler blocks later)
        nc.sync.dma_start(wq_sb[:, :, 128:512], wrearr(wq[:, 128:512]))
        nc.sync.dma_start(wk_sb[:, :, 128:512], wrearr(wk[:, 128:512]))
        nc.sync.dma_start(wo_sb[:], wo.rearrange("(ec p) d -> p ec d", p=128))
        nc.sync.dma_start(bo_sb[:], bod[:])

        # ---- filler queues ----
        done = {("q", 0, 0), ("k", 0, 0)}

        def fill_thunk(key):
            def f():
                if key in done:
                    return
                kind = key[0]
                if kind == "v":
                    v_block(key[1], key[2])
                else:
                    qk_block(*key)
                done.add(key)
            return f

        fillq = deque()
        for sweep in range(SC):
            for g in range(G):
                if (g, sweep) != (0, 0):
                    fillq.append(("q", g, sweep))
                    fillq.append(("k", g, sweep))
            for qi in range(4):
                fillq.append(("v", sweep, qi))

        opq = deque()  # (dc, sc) output projection blocks
        epi_done = [0] * SC  # epilogues emitted per sweep

        def op_ready():
            return opq and epi_done[opq[0][1]] >= G

        def emit_fill(reserve_ops, margin=0.0):
            # top up the PE queue until it is `margin` ns ahead of ScalarE.
            # proj blocks first: their x loads issue on the in-order sync
            # queue, and a result-dependent out-store DMA ahead of them
            # would stall the loads until the attention pipeline catches up
            while acc["pe"] < acc["sc"] + margin:
                if fillq:
                    fill_thunk(fillq.popleft())()
                elif op_ready() and len(opq) > reserve_ops:
                    dc, sc = opq.popleft()
                    op_block(dc, sc)
                else:
                    return False
            return True

        def hard_prep(keys):
            for key in keys:
                if key not in done:
                    fill_thunk(key)()
                    if key in fillq:
                        fillq.remove(key)

        # ---- attention units ----
        units = [(g4, g) for g4 in range(SC) for g in range(G)]

        def emit_unit(g4, g, prev, last):
            """Scores+exp+mask for unit (g4, g); AV+normalize of prev unit
            interleaved into this unit's windows; one filler per window."""
            blocks, off, cols, wins = g4_geom(g4)
            pts = [
                pt_pool.tile(
                    [128, cols], bf16, tag=f"pts{j}", name=f"pt{g4}_{g}_{j}"
                )
                for j in range(2)
            ]
            carry = deque()
            if prev is not None:
                carry.extend(av_plan(*prev))
            for wi, win in enumerate(wins):
                wbase = off[win[0][0]]
                wcols = sum(b[2] for b in win)
                stt = [
                    ps_st.tile(
                        [128, 1024], f32, tag="st", name=f"st{g4}_{g}_{wi}_{j}"
                    )
                    for j in range(2)
                ]
                # one start/stop pair per 512-col PSUM bank: start marks the
                # whole bank pending-zero, so only the first matmul touching
                # a bank sets it; later ones overwrite via the pending mask
                ps, p = [], 0
                for _, _, n in win:
                    ps.append(p)
                    p += n
                bank_of = {}
                for bi, p0 in enumerate(ps):
                    bank_of.setdefault(p0 // 512, []).append(bi)
                for j in range(2):
                    ro = j * HD
                    for bi, (t, q_lo, n) in enumerate(win):
                        blist = bank_of[ps[bi] // 512]
                        nc.tensor.matmul(
                            stt[j][:, ps[bi] : ps[bi] + n],
                            kT[ro : ro + HD, g, t * 128 : t * 128 + 128],
                            qT[ro : ro + HD, g, q_lo : q_lo + n],
                            start=(bi == blist[0]),
                            stop=(bi == blist[-1]),
                        )
                for j in range(2):
                    nc.scalar.activation(
                        pts[j][:, wbase : wbase + wcols],
                        stt[j][:, 0:wcols],
                        Exp,
                        scale=1.0 / np.sqrt(HD),
                    )
                acc["pe"] += wcols * 2 * 0.41666
                acc["sc"] += wcols * 2 * 0.833 + 560
                for t, q_lo, n in win:
                    if t >= 4 * g4:  # diagonal tile: mask first 128 cols
                        for j in range(2):
                            nc.vector.tensor_tensor(
                                pts[j][:, off[t] : off[t] + 128],
                                pts[j][:, off[t] : off[t] + 128],
                                mask_sb[:],
                                mult,
                            )
                if carry:
                    carry.popleft()()
                # hold back ~6 outproj blocks: they are the only PE work
                # that can cover the final unit's normalize-chain latency
                emit_fill(6, margin=2500.0 if g4 == SC - 1 else 0.0)
            while carry:
                carry.popleft()()
            if prev is not None:
                emit_epilogue(*prev)
            return pts

        av_state = {}

        def av_plan(g4, g, pts):
            """AV thunks (one per window) + allocate the accumulators."""
            blocks, off, cols, wins = g4_geom(g4)
            av = [
                ps_av.tile([128, 512], f32, tag="av", name=f"av{g4}_{g}_{j}")
                for j in range(2)
            ]
            av_state[(g4, g)] = av
            thunks = []
            for wi, win in enumerate(wins):
                first = wi == 0
                final = wi == len(wins) - 1

                def emit_av(win=win, first=first, final=final):
                    for j in range(2):
                        h = 2 * g + j
                        for bi, (t, q_lo, n) in enumerate(win):
                            lo = q_lo - g4 * 512
                            nc.tensor.matmul(
                                av[j][0 : HD + 1, lo : lo + n],
                                v_sb[:, t, h, :],
                                pts[j][:, off[t] : off[t] + n],
                                start=(first and bi == 0),
                                stop=(final and bi == len(win) - 1),
                            )
                    acc["pe"] += sum(b[2] for b in win) * 2 * 0.41666
                thunks.append(emit_av)
            return thunks

        def emit_epilogue(g4, g, pts):
            """normalize: oT[:, g, g4 cols] = av[0:64] / av[64] (per column).
            Evacuate PSUM (frees the accumulator), reciprocal of the
            denominator row, partition broadcast, multiply - no DMA round
            trips (all partition offsets on quadrant boundaries)."""
            av = av_state.pop((g4, g))
            for j in range(2):
                sfx = f"{g4}_{g}_{j}"
                osb = ou_pool.tile([HD + 1, 512], f32, tag="ou", name=f"ou{sfx}")
                nc.vector.tensor_copy(osb[:], av[j][0 : HD + 1, :])
                # InstReciprocal cost is free-size driven (~6 passes), so a
                # [1,512] row costs 3.3us on one lane while the DMA-transposed
                # [128,4] layout costs ~130ns
                rs = r_pool.tile([128, 4], f32, tag="rs", name=f"rs{sfx}")
                nc.sync.dma_start(rs[:], osb[HD : HD + 1, :])
                rr = r_pool.tile([128, 4], f32, tag="rr", name=f"rr{sfx}")
                nc.vector.reciprocal(rr[:], rs[:])
                r1 = r_pool.tile([1, 512], f32, tag="r1", name=f"r1{sfx}")
                nc.sync.dma_start(r1[:], rr[:])
                r64 = r_pool.tile([HD, 512], f32, tag="rb", name=f"rb{sfx}")
                nc.gpsimd.partition_broadcast(r64[:], r1[:])
                nc.vector.tensor_tensor(
                    oT[j * HD : (j + 1) * HD, g, g4 * 512 : (g4 + 1) * 512],
                    osb[0:HD, :],
                    r64[:],
                    mult,
                )
            epi_done[g4] += 1
            if epi_done[g4] == G:
                for dc in range(D // 128):
                    opq.append((dc, g4))

        prev = None
        for i, (g4, g) in enumerate(units):
            need = [("q", g, g4), ("k", g, g4)]
            if prev is not None:
                need += [("v", prev[0], qi) for qi in range(4)]
            hard_prep(need)
            # prefetch next sweep's x slices so proj blocks at the sweep
            # boundary never wait on a just-issued DMA. Ring-rotation safety:
            # a slice may only be (re)allocated after every consumer of the
            # slot it displaces has been emitted, so sweep 0 prefetches all
            # three at its last unit, later sweeps stagger v at g==2.
            if g4 + 1 < SC:
                if g4 == 0:
                    if g == 3:
                        for kind in ("v", "q", "k"):
                            get_x(kind, 1)
                else:
                    if g == 2:
                        get_x("v", g4 + 1)
                    elif g == 3:
                        get_x("q", g4 + 1)
                        get_x("k", g4 + 1)
            pts = emit_unit(g4, g, prev, last=(i == len(units) - 1))
            prev = (g4, g, pts)

        # last unit: AV + normalize + the final output projection chunk
        hard_prep([("v", SC - 1, qi) for qi in range(4)])
        for thunk in av_plan(*prev):
            thunk()
            emit_fill(6, margin=2500.0)
        emit_epilogue(*prev)
        while fillq:
            fill_thunk(fillq.popleft())()
        while opq:
            dc, sc = opq.popleft()
            op_block(dc, sc)

    nc.compile()
    return nc


def core_inputs(queries, keys, values, Wq, bq, Wk, bk, Wv, bv, Wo, bo, b, hh):
    """Build the per-core input map (host-side sharding + bf16 cast)."""
    D = queries.shape[2]
    hs = slice(hh * NH, hh * NH + NH)

    def xt(x):
        return np.ascontiguousarray(x[b].astype(BF16).T)

    def wcat(W):
        return np.ascontiguousarray(
            np.transpose(W[hs], (1, 0, 2)).reshape(D, NH * HD).astype(BF16)
        )

    def bstack(bias):
        return np.ascontiguousarray(
            bias[hs].reshape(G, 128).T.astype(np.float32)
        )

    mask = np.triu(np.ones((128, 128), np.float32)).astype(BF16)
    return {
        "xq": xt(queries),
        "xk": xt(keys),
        "xv": xt(values),
        "wq": wcat(Wq),
        "wk": wcat(Wk),
        "wv": wcat(Wv),
        "wo": np.ascontiguousarray(Wo[hh * NH * HD : (hh + 1) * NH * HD].astype(BF16)),
        "bq": bstack(bq),
        "bk": bstack(bk),
        "bv": np.ascontiguousarray(
            np.broadcast_to(bv[hs].reshape(1, NH, HD), (128, NH, HD)).astype(np.float32)
        ),
        "bo": np.ascontiguousarray(
            (bo.reshape(D // 128, 128) / 2.0).T.astype(np.float32)
        ),
        "mask": mask,
    }


_NC_CACHE = {}


def _get_nc(S, D):
    key = (S, D)
    if key not in _NC_CACHE:
        _NC_CACHE[key] = build_nc(S, D)
    return _NC_CACHE[key]


def kernel(keys, queries, values, Wq, bq, Wk, bk, Wv, bv, Wo, bo, _trace=False):
    keys, queries, values = (np.asarray(a) for a in (keys, queries, values))
    Wq, bq, Wk, bk, Wv, bv, Wo, bo = (
        np.asarray(a) for a in (Wq, bq, Wk, bk, Wv, bv, Wo, bo)
    )
    B, S, D = queries.shape
    nc = _get_nc(S, D)

    in_maps = [
        core_inputs(queries, keys, values, Wq, bq, Wk, bk, Wv, bv, Wo, bo, c // 2, c % 2)
        for c in range(8)
    ]
    from concourse.bass_utils import run_bass_kernel_spmd

    res = run_bass_kernel_spmd(
        nc, in_maps, core_ids=list(range(8)), trace=_trace
    )
    kernel.last_result = res
    outs = [r["out"] for r in res.results]
    out = np.empty((B, S, D), np.float32)
    for b in range(B):
        out[b] = (outs[2 * b] + outs[2 * b + 1]).T
    return out


# revision 37
# speedup vs baseline: 1.1475x; 1.1475x over previous
"""Multi-head causal attention (B=4, S=2048, D=1024, H=16) on 8 Trainium2 cores.

Sharding: core c -> (batch b = c//2, head-half hh = c%2): each core computes
attention for one batch and 8 of the 16 heads plus the partial output
projection against its row-shard of Wo. Host sums the per-batch core pair
(the Wo row-shard all-reduce) and transposes.

On-device schedule (per core; matmul operands bf16, accumulation fp32):
  - scores are computed transposed (S^T tiles: keys on partitions) so exp on
    ScalarE writes P^T directly; softmax denominators come free from a
    ones-column appended to V (row 64 of the P^T @ V_aug product).
  - work is organized as 16 "units" = (query-group g4 of 512 tokens) x
    (head-pair g), swept g4-major. Each unit runs windows of <=1024 score
    columns: PE score matmuls -> ScalarE exp -> (diag-tile mask on DVE).
  - the AV accumulation of unit i is interleaved between the score windows
    of unit i+1, and projection / output-projection blocks are fed one per
    window as PE filler, so the PE never idles waiting for ScalarE (idle PE
    drops out of its high p-state and halves matmul throughput).
  - softmax normalization: reciprocal of the denominator row on DVE,
    partition-broadcast on GpSimd, multiply on DVE - no DMA round trips.
  - output projection is emitted per (128-row d chunk, 512-col s chunk) as
    soon as the oT columns for that s chunk are complete, keeping only the
    last s-group's projection on the critical-path tail.
"""

import os
import sys
from collections import deque
from contextlib import ExitStack

for _p in (
    "/opt/trn_rl_repo/concourse",
    "/root/.axon_site/_ro/trn_rl_repo/concourse",
):
    if os.path.isdir(_p) and _p not in sys.path:
        sys.path.append(_p)

import numpy as np
import ml_dtypes

BF16 = ml_dtypes.bfloat16

HD = 64          # head dim
NH = 8           # heads per core
G = NH // 2      # head-pair groups (2 heads -> 128 partitions)
EC = NH * HD // 128  # o^T feature chunks (=4)


def g4_geom(g4):
    """Blocks/windows for query group g4 (queries [512*g4, 512*(g4+1))).

    blocks: (t, q_lo, n) - key tile t contributes P^T columns for queries
    [q_lo, q_lo+n). off[t]: column offset of tile t's block in the packed
    pts tile. Windows greedily pack consecutive blocks to <=1024 columns;
    block offsets within a window never straddle a 512 (PSUM bank) boundary
    by construction (full 512 blocks, then a 512/384/256/128 tail).
    """
    blocks = []
    for t in range(4 * g4 + 4):
        if t < 4 * g4:
            q_lo, n = g4 * 512, 512
        else:
            q_lo = t * 128
            n = (g4 + 1) * 512 - q_lo
        blocks.append((t, q_lo, n))
    off = {}
    c = 0
    for t, _, n in blocks:
        off[t] = c
        c += n
    # greedy-pack consecutive blocks into <=1024-col windows (fewer, larger
    # exp instructions); score matmuls use one start/stop pair per PSUM bank
    wins = []
    cur, cc = [], 0
    for b in blocks:
        if cc + b[2] > 1024:
            wins.append(cur)
            cur, cc = [], 0
        cur.append(b)
        cc += b[2]
    if cur:
        wins.append(cur)
    return blocks, off, c, wins


def build_nc(S, D):
    import concourse.tile as tile
    from concourse import bacc, mybir

    f32 = mybir.dt.float32
    bf16 = mybir.dt.bfloat16
    Exp = mybir.ActivationFunctionType.Exp
    add = mybir.AluOpType.add
    mult = mybir.AluOpType.mult

    KC = D // 128    # contraction chunks over model dim
    ST = S // 128    # 128-token tiles
    SC = S // 512    # 512-token chunks

    nc = bacc.Bacc(None, target_bir_lowering=False)

    xq = nc.dram_tensor("xq", [D, S], bf16, kind="ExternalInput")
    xk = nc.dram_tensor("xk", [D, S], bf16, kind="ExternalInput")
    xv = nc.dram_tensor("xv", [D, S], bf16, kind="ExternalInput")
    wq = nc.dram_tensor("wq", [D, NH * HD], bf16, kind="ExternalInput")
    wk = nc.dram_tensor("wk", [D, NH * HD], bf16, kind="ExternalInput")
    wv = nc.dram_tensor("wv", [D, NH * HD], bf16, kind="ExternalInput")
    wo = nc.dram_tensor("wo", [NH * HD, D], bf16, kind="ExternalInput")
    bqd = nc.dram_tensor("bq", [128, G], f32, kind="ExternalInput")
    bkd = nc.dram_tensor("bk", [128, G], f32, kind="ExternalInput")
    bvd = nc.dram_tensor("bv", [128, NH, HD], f32, kind="ExternalInput")
    bod = nc.dram_tensor("bo", [128, D // 128], f32, kind="ExternalInput")
    maskd = nc.dram_tensor("mask", [128, 128], bf16, kind="ExternalInput")
    out = nc.dram_tensor("out", [D, S], f32, kind="ExternalOutput")

    with tile.TileContext(nc) as tc, ExitStack() as ctx:
        const_pool = ctx.enter_context(tc.tile_pool(name="const", bufs=1))
        xpool = ctx.enter_context(tc.tile_pool(name="x", bufs=3))
        qk_pool = ctx.enter_context(tc.tile_pool(name="qk", bufs=1))
        v_pool = ctx.enter_context(tc.tile_pool(name="v", bufs=1))
        pt_pool = ctx.enter_context(tc.tile_pool(name="pt", bufs=2))
        o_pool = ctx.enter_context(tc.tile_pool(name="o", bufs=1))
        r_pool = ctx.enter_context(tc.tile_pool(name="r", bufs=2))
        ou_pool = ctx.enter_context(tc.tile_pool(name="ou", bufs=2))
        ps_st = ctx.enter_context(tc.tile_pool(name="psst", bufs=2, space="PSUM"))
        ps_av = ctx.enter_context(tc.tile_pool(name="psav", bufs=2, space="PSUM"))
        ps_sm = ctx.enter_context(tc.tile_pool(name="pssm", bufs=2, space="PSUM"))

        # ---- persistent SBUF tensors ----
        wq_sb = const_pool.tile([128, KC, NH * HD], bf16)
        wk_sb = const_pool.tile([128, KC, NH * HD], bf16)
        wv_sb = const_pool.tile([128, KC, NH * HD], bf16)
        wo_sb = const_pool.tile([128, EC, D], bf16)
        bq_sb = const_pool.tile([128, G], f32)
        bk_sb = const_pool.tile([128, G], f32)
        bv_sb = const_pool.tile([128, NH, HD], f32)
        bo_sb = const_pool.tile([128, D // 128], f32)
        mask_sb = const_pool.tile([128, 128], bf16)

        qT = qk_pool.tile([128, G, S], bf16, tag="qT")
        kT = qk_pool.tile([128, G, S], bf16, tag="kT")
        v_sb = v_pool.tile([128, ST, NH, HD + 1], bf16, tag="v")
        oT = o_pool.tile([128, EC, S], bf16, tag="oT")

        # ---- startup DMAs: pair-0 q/k weights + first x slices first ----
        def wrearr(w):
            return w.rearrange("(kc p) m -> p kc m", p=128)

        nc.sync.dma_start(wq_sb[:, :, 0:128], wrearr(wq[:, 0:128]))
        xcache = {}

        def get_x(kind, sc):
            key = (kind, sc)
            if key not in xcache:
                xt = xpool.tile([128, KC, 512], bf16, tag="xb", name=f"x{kind}{sc}")
                src = {"q": xq, "k": xk, "v": xv}[kind]
                nc.sync.dma_start(
                    xt[:],
                    src.rearrange("(kc p) s -> p kc s", p=128)[
                        :, :, sc * 512 : (sc + 1) * 512
                    ],
                )
                xcache[key] = xt
            return xcache[key]

        get_x("q", 0)
        nc.sync.dma_start(bq_sb[:], bqd[:])

        # ones columns of V_aug (softmax denominator trick)
        nc.vector.memset(v_sb[:, :, :, HD : HD + 1], 1.0)

        # emitted-work accounting (ns) for deficit-paced filler: keep the PE
        # queue fed just ahead of the ScalarE exp pace, so early windows don't
        # over-consume filler and starve the late sweeps into p-state drops
        acc = {"pe": 0.0, "sc": 0.0}

        def qk_block(kind, g, sc):
            """One 512-col projection chunk for pair g of q or k."""
            xt = get_x(kind, sc)
            wsb = wq_sb if kind == "q" else wk_sb
            dst = qT if kind == "q" else kT
            bsb = bq_sb if kind == "q" else bk_sb
            psum = ps_sm.tile([128, 512], f32, tag="mm", name=f"p{kind}{g}_{sc}")
            for c in range(KC):
                nc.tensor.matmul(
                    psum[:],
                    wsb[:, c, g * 128 : (g + 1) * 128],
                    xt[:, c, :],
                    start=(c == 0),
                    stop=(c == KC - 1),
                )
            nc.vector.tensor_scalar_add(
                dst[:, g, sc * 512 : (sc + 1) * 512], psum[:], bsb[:, g : g + 1]
            )
            acc["pe"] += 1750

        def v_block(sc, qi):
            """V projection for tokens [sc*512 + qi*128, +128)."""
            xt = get_x("v", sc)
            psum = ps_sm.tile([128, NH, HD], f32, tag="mm", name=f"pv{sc}_{qi}")
            for c in range(KC):
                nc.tensor.matmul(
                    psum[:],
                    xt[:, c, qi * 128 : (qi + 1) * 128],
                    wv_sb[:, c, :],
                    start=(c == 0),
                    stop=(c == KC - 1),
                )
            nc.vector.tensor_tensor(
                v_sb[:, sc * 4 + qi, :, 0:HD], psum[:], bv_sb[:], add
            )
            acc["pe"] += 1750

        def op_block(dc, sc):
            """Output projection chunk out[dc*128:+128, sc*512:+512]."""
            psum = ps_sm.tile([128, 512], f32, tag="mm", name=f"po{dc}_{sc}")
            for c in range(EC):
                nc.tensor.matmul(
                    psum[:],
                    wo_sb[:, c, dc * 128 : (dc + 1) * 128],
                    oT[:, c, sc * 512 : (sc + 1) * 512],
                    start=(c == 0),
                    stop=(c == EC - 1),
                )
            ot = ou_pool.tile([128, 512], f32, tag="ot", name=f"ot{dc}_{sc}")
            nc.vector.tensor_scalar_add(ot[:], psum[:], bo_sb[:, dc : dc + 1])
            nc.sync.dma_start(
                out[dc * 128 : (dc + 1) * 128, sc * 512 : (sc + 1) * 512], ot[:]
            )
            acc["pe"] += 880

        # upfront: pair-0 q/k projection for the first query/key chunk
        qk_block("q", 0, 0)
        nc.sync.dma_start(wk_sb[:, :, 0:128], wrearr(wk[:, 0:128]))
        get_x("k", 0)
        nc.sync.dma_start(bk_sb[:], bkd[:])
        # V-path loads early: the first AV (carried into unit (0,1)) needs
        # the sweep-0 V projection done by ~t+10us
        nc.sync.dma_start(wv_sb[:], wrearr(wv))
        get_x("v", 0)
        nc.sync.dma_start(bv_sb[:], bvd[:])
        qk_block("k", 0, 0)
        nc.sync.dma_start(mask_sb[:], maskd[:])
        # remaining weight/bias loads (consumed by filler blocks later)
        nc.sync.dma_start(wq_sb[:, :, 128:512], wrearr(wq[:, 128:512]))
        nc.sync.dma_start(wk_sb[:, :, 128:512], wrearr(wk[:, 128:512]))
        nc.sync.dma_start(wo_sb[:], wo.rearrange("(ec p) d -> p ec d", p=128))
        nc.sync.dma_start(bo_sb[:], bod[:])

        # ---- filler queues ----
        done = {("q", 0, 0), ("k", 0, 0)}

        def fill_thunk(key):
            def f():
                if key in done:
                    return
                kind = key[0]
                if kind == "v":
                    v_block(key[1], key[2])
                else:
                    qk_block(*key)
                done.add(key)
            return f

        fillq = deque()
        for sweep in range(SC):
            for g in range(G):
                if (g, sweep) != (0, 0):
                    fillq.append(("q", g, sweep))
                    fillq.append(("k", g, sweep))
            for qi in range(4):
                fillq.append(("v", sweep, qi))

        opq = deque()  # (dc, sc) output projection blocks
        epi_done = [0] * SC  # epilogues emitted per sweep

        def op_ready():
            return opq and epi_done[opq[0][1]] >= G

        def emit_fill(reserve_ops, margin=0.0):
            # one filler block per window keeps the PE queue deep (a shallow
            # queue exposes every semaphore latency). proj blocks first:
            # their x loads issue on the in-order sync queue, and a
            # result-dependent out-store DMA ahead of them would stall the
            # loads until the attention pipeline catches up
            if fillq:
                fill_thunk(fillq.popleft())()
                return True
            if op_ready() and len(opq) > reserve_ops:
                dc, sc = opq.popleft()
                op_block(dc, sc)
                return True
            return False

        def hard_prep(keys):
            for key in keys:
                if key not in done:
                    fill_thunk(key)()
                    if key in fillq:
                        fillq.remove(key)

        # ---- attention units ----
        units = [(g4, g) for g4 in range(SC) for g in range(G)]

        def emit_unit(g4, g, prev, last):
            """Scores+exp+mask for unit (g4, g); AV+normalize of prev unit
            interleaved into this unit's windows; one filler per window."""
            blocks, off, cols, wins = g4_geom(g4)
            pts = [
                pt_pool.tile(
                    [128, cols], bf16, tag=f"pts{j}", name=f"pt{g4}_{g}_{j}"
                )
                for j in range(2)
            ]
            carry = deque()
            if prev is not None:
                carry.extend(av_plan(*prev))
            for wi, win in enumerate(wins):
                wbase = off[win[0][0]]
                wcols = sum(b[2] for b in win)
                stt = [
                    ps_st.tile(
                        [128, 1024], f32, tag="st", name=f"st{g4}_{g}_{wi}_{j}"
                    )
                    for j in range(2)
                ]
                # one start/stop pair per 512-col PSUM bank: start marks the
                # whole bank pending-zero, so only the first matmul touching
                # a bank sets it; later ones overwrite via the pending mask
                ps, p = [], 0
                for _, _, n in win:
                    ps.append(p)
                    p += n
                bank_of = {}
                for bi, p0 in enumerate(ps):
                    bank_of.setdefault(p0 // 512, []).append(bi)
                for j in range(2):
                    ro = j * HD
                    for bi, (t, q_lo, n) in enumerate(win):
                        blist = bank_of[ps[bi] // 512]
                        nc.tensor.matmul(
                            stt[j][:, ps[bi] : ps[bi] + n],
                            kT[ro : ro + HD, g, t * 128 : t * 128 + 128],
                            qT[ro : ro + HD, g, q_lo : q_lo + n],
                            start=(bi == blist[0]),
                            stop=(bi == blist[-1]),
                        )
                for j in range(2):
                    nc.scalar.activation(
                        pts[j][:, wbase : wbase + wcols],
                        stt[j][:, 0:wcols],
                        Exp,
                        scale=1.0 / np.sqrt(HD),
                    )
                acc["pe"] += wcols * 2 * 0.41666
                acc["sc"] += wcols * 2 * 0.833 + 560
                for t, q_lo, n in win:
                    if t >= 4 * g4:  # diagonal tile: mask first 128 cols
                        for j in range(2):
                            nc.vector.tensor_tensor(
                                pts[j][:, off[t] : off[t] + 128],
                                pts[j][:, off[t] : off[t] + 128],
                                mask_sb[:],
                                mult,
                            )
                if carry:
                    carry.popleft()()
                # hold back a few outproj blocks: they are the only PE work
                # that can cover the final unit's normalize-chain latency
                emit_fill(0 if last else 3)
            while carry:
                carry.popleft()()
            if prev is not None:
                emit_epilogue(*prev)
            return pts

        av_state = {}

        def av_plan(g4, g, pts):
            """AV thunks (one per window) + allocate the accumulators."""
            blocks, off, cols, wins = g4_geom(g4)
            av = [
                ps_av.tile([128, 512], f32, tag="av", name=f"av{g4}_{g}_{j}")
                for j in range(2)
            ]
            av_state[(g4, g)] = av
            thunks = []
            for wi, win in enumerate(wins):
                first = wi == 0
                final = wi == len(wins) - 1

                def emit_av(win=win, first=first, final=final):
                    for j in range(2):
                        h = 2 * g + j
                        for bi, (t, q_lo, n) in enumerate(win):
                            lo = q_lo - g4 * 512
                            nc.tensor.matmul(
                                av[j][0 : HD + 1, lo : lo + n],
                                v_sb[:, t, h, :],
                                pts[j][:, off[t] : off[t] + n],
                                start=(first and bi == 0),
                                stop=(final and bi == len(win) - 1),
                            )
                    acc["pe"] += sum(b[2] for b in win) * 2 * 0.41666
                thunks.append(emit_av)
            return thunks

        def emit_epilogue(g4, g, pts):
            """normalize: oT[:, g, g4 cols] = av[0:64] / av[64] (per column).
            Evacuate PSUM (frees the accumulator), reciprocal of the
            denominator row, partition broadcast, multiply - no DMA round
            trips (all partition offsets on quadrant boundaries)."""
            av = av_state.pop((g4, g))
            for j in range(2):
                sfx = f"{g4}_{g}_{j}"
                osb = ou_pool.tile([HD + 1, 512], f32, tag="ou", name=f"ou{sfx}")
                nc.vector.tensor_copy(osb[:], av[j][0 : HD + 1, :])
                # InstReciprocal cost is free-size driven (~6 passes), so a
                # [1,512] row costs 3.3us on one lane while the DMA-transposed
                # [128,4] layout costs ~130ns
                rs = r_pool.tile([128, 4], f32, tag="rs", name=f"rs{sfx}")
                nc.sync.dma_start(rs[:], osb[HD : HD + 1, :])
                rr = r_pool.tile([128, 4], f32, tag="rr", name=f"rr{sfx}")
                nc.vector.reciprocal(rr[:], rs[:])
                r1 = r_pool.tile([1, 512], f32, tag="r1", name=f"r1{sfx}")
                nc.sync.dma_start(r1[:], rr[:])
                r64 = r_pool.tile([HD, 512], f32, tag="rb", name=f"rb{sfx}")
                nc.gpsimd.partition_broadcast(r64[:], r1[:])
                nc.vector.tensor_tensor(
                    oT[j * HD : (j + 1) * HD, g, g4 * 512 : (g4 + 1) * 512],
                    osb[0:HD, :],
                    r64[:],
                    mult,
                )
            epi_done[g4] += 1
            if epi_done[g4] == G:
                for dc in range(D // 128):
                    opq.append((dc, g4))

        prev = None
        for i, (g4, g) in enumerate(units):
            need = [("q", g, g4), ("k", g, g4)]
            if prev is not None:
                need += [("v", prev[0], qi) for qi in range(4)]
            hard_prep(need)
            # prefetch next sweep's x slices so proj blocks at the sweep
            # boundary never wait on a just-issued DMA. Ring-rotation safety:
            # a slice may only be (re)allocated after every consumer of the
            # slot it displaces has been emitted, so sweep 0 prefetches all
            # three at its last unit, later sweeps stagger v at g==2.
            if g4 + 1 < SC:
                if g4 == 0:
                    if g == 3:
                        for kind in ("v", "q", "k"):
                            get_x(kind, 1)
                else:
                    if g == 2:
                        get_x("v", g4 + 1)
                    elif g == 3:
                        get_x("q", g4 + 1)
                        get_x("k", g4 + 1)
            pts = emit_unit(g4, g, prev, last=(i == len(units) - 1))
            prev = (g4, g, pts)

        # last unit: AV + normalize + the final output projection chunk
        hard_prep([("v", SC - 1, qi) for qi in range(4)])
        for thunk in av_plan(*prev):
            thunk()
            emit_fill(0)
        emit_epilogue(*prev)
        while fillq:
            fill_thunk(fillq.popleft())()
        while opq:
            dc, sc = opq.popleft()
            op_block(dc, sc)

    nc.compile()
    return nc


def core_inputs(queries, keys, values, Wq, bq, Wk, bk, Wv, bv, Wo, bo, b, hh):
    """Build the per-core input map (host-side sharding + bf16 cast)."""
    D = queries.shape[2]
    hs = slice(hh * NH, hh * NH + NH)

    def xt(x):
        return np.ascontiguousarray(x[b].astype(BF16).T)

    def wcat(W):
        return np.ascontiguousarray(
            np.transpose(W[hs], (1, 0, 2)).reshape(D, NH * HD).astype(BF16)
        )

    def bstack(bias):
        return np.ascontiguousarray(
            bias[hs].reshape(G, 128).T.astype(np.float32)
        )

    mask = np.triu(np.ones((128, 128), np.float32)).astype(BF16)
    return {
        "xq": xt(queries),
        "xk": xt(keys),
        "xv": xt(values),
        "wq": wcat(Wq),
        "wk": wcat(Wk),
        "wv": wcat(Wv),
        "wo": np.ascontiguousarray(Wo[hh * NH * HD : (hh + 1) * NH * HD].astype(BF16)),
        "bq": bstack(bq),
        "bk": bstack(bk),
        "bv": np.ascontiguousarray(
            np.broadcast_to(bv[hs].reshape(1, NH, HD), (128, NH, HD)).astype(np.float32)
        ),
        "bo": np.ascontiguousarray(
            (bo.reshape(D // 128, 128) / 2.0).T.astype(np.float32)
        ),
        "mask": mask,
    }


_NC_CACHE = {}


def _get_nc(S, D):
    key = (S, D)
    if key not in _NC_CACHE:
        _NC_CACHE[key] = build_nc(S, D)
    return _NC_CACHE[key]


def kernel(keys, queries, values, Wq, bq, Wk, bk, Wv, bv, Wo, bo, _trace=False):
    keys, queries, values = (np.asarray(a) for a in (keys, queries, values))
    Wq, bq, Wk, bk, Wv, bv, Wo, bo = (
        np.asarray(a) for a in (Wq, bq, Wk, bk, Wv, bv, Wo, bo)
    )
    B, S, D = queries.shape
    nc = _get_nc(S, D)

    in_maps = [
        core_inputs(queries, keys, values, Wq, bq, Wk, bk, Wv, bv, Wo, bo, c // 2, c % 2)
        for c in range(8)
    ]
    from concourse.bass_utils import run_bass_kernel_spmd

    res = run_bass_kernel_spmd(
        nc, in_maps, core_ids=list(range(8)), trace=_trace
    )
    kernel.last_result = res
    outs = [r["out"] for r in res.results]
    out = np.empty((B, S, D), np.float32)
    for b in range(B):
        out[b] = (outs[2 * b] + outs[2 * b + 1]).T
    return out
